# revision 1
# baseline (speedup 1.0000x reference)
"""Trainium2 Bass kernel: single-layer causal attention block (q/k/v/o + RoPE).

Sharding: 8 cores = 2 batches x 4 head-groups (4 heads each).
Per core (SPMD, differs only in input data):
  - inputs: x[b]^T, per-head-group weight slices, RoPE tables, causal masks
  - compute: q/k/v projections (transposed layouts, fp32r matmuls), RoPE via
    even/odd permuted layout, scores^T = K^T-stationary matmul, exp on ACT,
    multiplicative causal mask on diagonal tiles, PV with ones-column rowsums,
    partial o_proj output [2048, 1024].
Host: sums the 4 per-head-group partials per batch (row-sharded o_proj
unshard) and stacks the 2 batches.
"""

import os
import sys

import numpy as np

sys.path.insert(0, "/opt/trn_rl_repo")

import concourse.bass as bass  # noqa: E402
import concourse.tile as tile  # noqa: E402
from concourse import bacc, mybir  # noqa: E402
from concourse import bass_utils  # noqa: E402

B, S, D, H, DK = 2, 2048, 1024, 16, 64
NCORES = 8
HPC = H // 4  # 4 heads per core
CW = HPC * DK  # 256 head-dim columns per core
VW = DK + 1  # 65: v width per head incl ones column
ND = D // 128  # 8 contraction chunks
NS = S // 128  # 16 s-tiles
NSC = S // 512  # 4 s-chunks
ROPE_THETA = 10000.0

F32 = mybir.dt.float32
F32R = mybir.dt.float32r
EXP = mybir.ActivationFunctionType.Exp


def _build_kernel(tc, nc, xt, wq, wk, wv, wo, cs, sn, mk, ones, out):
    from contextlib import ExitStack
    _stack = ExitStack()
    constp = _stack.enter_context(tc.tile_pool(name="const", bufs=1))
    pers = _stack.enter_context(tc.tile_pool(name="persist", bufs=1))

    wq_sb = constp.tile([128, ND * CW], F32R)
    wk_sb = constp.tile([128, ND * CW], F32R)
    wv_sb = constp.tile([128, ND * CW], F32R)
    wo_sb = constp.tile([128, 2 * D], F32R)
    cs_sb = constp.tile([128, S], F32)
    sn_sb = constp.tile([128, S], F32)
    mk_sb = constp.tile([128, 4 * 512], F32)
    for d in range(ND):
        nc.sync.dma_start(wq_sb[:, d * CW:(d + 1) * CW], wq[d * 128:(d + 1) * 128, :])
        nc.sync.dma_start(wk_sb[:, d * CW:(d + 1) * CW], wk[d * 128:(d + 1) * 128, :])
        nc.sync.dma_start(wv_sb[:, d * CW:(d + 1) * CW], wv[d * 128:(d + 1) * 128, :])
    nc.sync.dma_start(wo_sb[:, 0:D], wo[0:128, :])
    nc.sync.dma_start(wo_sb[:, D:2 * D], wo[128:256, :])
    nc.sync.dma_start(cs_sb[:], cs[:])
    nc.sync.dma_start(sn_sb[:], sn[:])
    nc.sync.dma_start(mk_sb[:], mk[:])

    qe_sb = pers.tile([128, S], F32R)
    qo_sb = pers.tile([128, S], F32R)
    ke_sb = pers.tile([128, S], F32R)
    ko_sb = pers.tile([128, S], F32R)
    v_sb = pers.tile([128, NS * HPC * VW], F32R)
    ctx_sb = pers.tile([128, 2 * S], F32R)

    # ---- Phase 1: projections ----
    with tc.tile_pool(name="xt", bufs=1) as xtp, \
         tc.tile_pool(name="pjps", bufs=5, space="PSUM") as pjps, \
         tc.tile_pool(name="vps", bufs=2, space="PSUM") as vps:
        xts = xtp.tile([128, ND * S], F32R)
        for d in range(ND):
            nc.sync.dma_start(xts[:, d * S:(d + 1) * S], xt[d * 128:(d + 1) * 128, :])
        blocks = [(qe_sb, wq_sb, 0), (qo_sb, wq_sb, 128),
                  (ke_sb, wk_sb, 0), (ko_sb, wk_sb, 128)]
        for dst, wsb, co in blocks:
            ps = [pjps.tile([128, 512], F32, name=f"pjt{_i}", tag="pjt") for _i in range(NSC)]
            for d in range(ND):
                lhsT = wsb[:, d * CW + co: d * CW + co + 128]
                for sc in range(NSC):
                    nc.tensor.matmul(
                        ps[sc][:],
                        lhsT,
                        xts[:, d * S + sc * 512: d * S + (sc + 1) * 512],
                        start=(d == 0), stop=(d == ND - 1))
            for sc in range(NSC):
                nc.vector.tensor_copy(dst[:, sc * 512:(sc + 1) * 512], ps[sc][:])
        for sm in range(NS):
            pv = vps.tile([128, CW], F32)
            for d in range(ND):
                nc.tensor.matmul(
                    pv[:],
                    xts[:, d * S + sm * 128: d * S + sm * 128 + 128],
                    wv_sb[:, d * CW:(d + 1) * CW],
                    start=(d == 0), stop=(d == ND - 1))
            base = sm * HPC * VW
            dst3 = v_sb[:, base:base + HPC * VW].rearrange("p (h c) -> p h c", c=VW)
            nc.vector.tensor_copy(dst3[:, :, 0:DK],
                                  pv[:].rearrange("p (h c) -> p h c", c=DK))
            nc.sync.dma_start(dst3[:, :, DK:DK + 1],
                              ones[:, sm * HPC:(sm + 1) * HPC].rearrange(
                                  "p (h c) -> p h c", c=1))

    # ---- Phase 2: RoPE (interleaved, on even/odd permuted layout) ----
    with tc.tile_pool(name="ropet", bufs=1) as rtp:
        for (e, o) in [(qe_sb, qo_sb), (ke_sb, ko_sb)]:
            t1 = rtp.tile([128, S], F32, tag="t1")
            t2 = rtp.tile([128, S], F32, tag="t2")
            t3 = rtp.tile([128, S], F32, tag="t3")
            t4 = rtp.tile([128, S], F32, tag="t4")
            nc.vector.tensor_mul(t1[:], e[:], cs_sb[:])
            nc.vector.tensor_mul(t2[:], o[:], sn_sb[:])
            nc.vector.tensor_mul(t3[:], o[:], cs_sb[:])
            nc.vector.tensor_mul(t4[:], e[:], sn_sb[:])
            nc.vector.tensor_sub(e[:], t1[:], t2[:])
            nc.vector.tensor_add(o[:], t3[:], t4[:])

    # ---- Phase 3: attention per head ----
    with tc.tile_pool(name="sps", bufs=4, space="PSUM") as sps, \
         tc.tile_pool(name="cps", bufs=2, space="PSUM") as cps, \
         tc.tile_pool(name="expool", bufs=6) as exp_pool, \
         tc.tile_pool(name="smp", bufs=4) as smp:
        for h in range(HPC):
            r0 = h * 32
            tp = (r0, 0)
            for c in range(NSC):
                nsk = 4 * (c + 1)
                pctx = cps.tile([VW, 512], F32)
                exps = []
                DEPTH = 3

                def pv_mm(t, nsk=nsk, pctx=pctx, exps=exps, h=h):
                    vbase = t * HPC * VW + h * VW
                    nc.tensor.matmul(
                        pctx[:],
                        v_sb[:, vbase:vbase + VW],
                        exps[t][:],
                        start=(t == 0), stop=(t == nsk - 1),
                        skip_group_check=True)

                for t in range(nsk):
                    pscore = sps.tile([128, 512], F32)
                    nc.tensor.matmul(
                        pscore[:],
                        ke_sb[r0:r0 + 32, t * 128:(t + 1) * 128],
                        qe_sb[r0:r0 + 32, c * 512:(c + 1) * 512],
                        start=True, stop=False, tile_position=tp,
                        skip_group_check=True)
                    nc.tensor.matmul(
                        pscore[:],
                        ko_sb[r0:r0 + 32, t * 128:(t + 1) * 128],
                        qo_sb[r0:r0 + 32, c * 512:(c + 1) * 512],
                        start=False, stop=True, tile_position=tp,
                        skip_group_check=True)
                    et = exp_pool.tile([128, 512], F32R)
                    nc.scalar.activation(et[:], pscore[:], EXP, scale=0.125)
                    j = t - 4 * c
                    if j >= 0:
                        nc.vector.tensor_mul(et[:], et[:],
                                             mk_sb[:, j * 512:(j + 1) * 512])
                    exps.append(et)
                    if t >= DEPTH:
                        pv_mm(t - DEPTH)
                for t in range(max(0, nsk - DEPTH), nsk):
                    pv_mm(t)
                rc = smp.tile([1, 512], F32, tag="rc")
                nc.vector.reciprocal(rc[:], pctx[DK:DK + 1, :])
                rb = smp.tile([64, 512], F32, tag="rb")
                nc.gpsimd.partition_broadcast(rb[:], rc[:])
                dst = ctx_sb[(h % 2) * 64:(h % 2) * 64 + 64,
                             (h // 2) * S + c * 512:(h // 2) * S + (c + 1) * 512]
                nc.vector.tensor_mul(dst, pctx[0:DK, :], rb[:])

    # ---- Phase 4: o_proj (partial over this core's 256 head dims) ----
    with tc.tile_pool(name="ops", bufs=4, space="PSUM") as opsp, \
         tc.tile_pool(name="obuf", bufs=4) as obp:
        for sm in range(NS):
            pos = [opsp.tile([128, 512], F32, name=f"opt{_i}", tag="opt") for _i in range(2)]
            for cb in range(2):
                lhsT = ctx_sb[:, cb * S + sm * 128: cb * S + sm * 128 + 128]
                for do_ in range(2):
                    nc.tensor.matmul(
                        pos[do_][:],
                        lhsT,
                        wo_sb[:, cb * D + do_ * 512: cb * D + (do_ + 1) * 512],
                        start=(cb == 0), stop=(cb == 1))
            for do_ in range(2):
                ot = obp.tile([128, 512], F32)
                nc.scalar.copy(ot[:], pos[do_][:])
                nc.sync.dma_start(out[sm * 128:(sm + 1) * 128,
                                      do_ * 512:(do_ + 1) * 512], ot[:])
    _stack.close()


def build_nc():
    nc = bacc.Bacc("TRN2", target_bir_lowering=False, debug=False,
                   enable_asserts=False, num_devices=NCORES)
    xt = nc.dram_tensor("xt", [D, S], F32R, kind="ExternalInput").ap()
    wq = nc.dram_tensor("wq", [D, CW], F32R, kind="ExternalInput").ap()
    wk = nc.dram_tensor("wk", [D, CW], F32R, kind="ExternalInput").ap()
    wv = nc.dram_tensor("wv", [D, CW], F32R, kind="ExternalInput").ap()
    wo = nc.dram_tensor("wo", [CW, D], F32R, kind="ExternalInput").ap()
    cs = nc.dram_tensor("cs", [128, S], F32, kind="ExternalInput").ap()
    sn = nc.dram_tensor("sn", [128, S], F32, kind="ExternalInput").ap()
    mk = nc.dram_tensor("mk", [128, 4 * 512], F32, kind="ExternalInput").ap()
    ones = nc.dram_tensor("ones", [128, NS * HPC], F32R,
                          kind="ExternalInput").ap()
    out = nc.dram_tensor("out_partial", [S, D], F32, kind="ExternalOutput").ap()
    with tile.TileContext(nc) as tc:
        _build_kernel(tc, nc, xt, wq, wk, wv, wo, cs, sn, mk, ones, out)
    nc.compile()
    return nc


def make_in_maps(in_features, q_proj_weight, k_proj_weight, v_proj_weight,
                 o_proj_weight, token_positions):
    x = np.asarray(in_features, dtype=np.float32)
    wq = np.asarray(q_proj_weight, dtype=np.float32)
    wk = np.asarray(k_proj_weight, dtype=np.float32)
    wv = np.asarray(v_proj_weight, dtype=np.float32)
    wo = np.asarray(o_proj_weight, dtype=np.float32)
    pos = np.asarray(token_positions).astype(np.float64)

    inv = ROPE_THETA ** (-2.0 * np.arange(DK // 2, dtype=np.float64) / DK)
    ang = inv[:, None] * pos[None, :]  # [32, S]
    cs_full = np.tile(np.cos(ang), (HPC, 1)).astype(np.float32)
    sn_full = np.tile(np.sin(ang), (HPC, 1)).astype(np.float32)

    p = np.arange(128)[:, None]
    f = np.arange(512)[None, :]
    mk = np.concatenate([(f >= j * 128 + p).astype(np.float32)
                         for j in range(4)], axis=1)

    in_maps = []
    for c in range(NCORES):
        b, g = c // 4, c % 4
        cols = np.arange(g * CW, (g + 1) * CW)
        hcols = cols.reshape(HPC, DK)
        qcols = np.concatenate([hcols[:, 0::2].reshape(-1),
                                hcols[:, 1::2].reshape(-1)])
        in_maps.append({
            "xt": np.ascontiguousarray(x[b].T),
            "wq": np.ascontiguousarray(wq[qcols, :].T),
            "wk": np.ascontiguousarray(wk[qcols, :].T),
            "wv": np.ascontiguousarray(wv[cols, :].T),
            "wo": np.ascontiguousarray(wo[:, cols].T),
            "cs": cs_full,
            "sn": sn_full,
            "mk": mk,
            "ones": np.ones((128, NS * HPC), np.float32),
        })
    return in_maps


_NC_CACHE = []
last_exec_ns = None


def kernel(in_features, q_proj_weight, k_proj_weight, v_proj_weight,
           o_proj_weight, token_positions, d_model=1024, num_heads=16,
           **_ignored):
    global last_exec_ns
    assert int(d_model) == D and int(num_heads) == H
    in_maps = make_in_maps(in_features, q_proj_weight, k_proj_weight,
                           v_proj_weight, o_proj_weight, token_positions)
    if not _NC_CACHE:
        _NC_CACHE.append(build_nc())
    nc = _NC_CACHE[0]
    trace = bool(int(os.environ.get("KERNEL_TRACE", "0")))
    res = bass_utils.run_bass_kernel_spmd(nc, in_maps,
                                          core_ids=list(range(NCORES)),
                                          trace=trace)
    last_exec_ns = res.exec_time_ns
    parts = [r["out_partial"].astype(np.float32) for r in res.results]
    out = np.stack([parts[0] + parts[1] + parts[2] + parts[3],
                    parts[4] + parts[5] + parts[6] + parts[7]])
    return out



# revision 4
# speedup vs baseline: 12806.2551x; 12806.2551x over previous
"""Trainium2 Bass kernel: single-layer causal attention block (q/k/v/o + RoPE).

Sharding: 8 cores = 2 batches x 4 head-groups (4 heads each).
Per core (SPMD, differs only in input data):
  - inputs: x[b]^T, per-head-group weight slices, RoPE tables, causal masks
  - compute: q/k/v projections (transposed layouts, fp32r matmuls), RoPE via
    even/odd permuted layout, scores^T = K^T-stationary matmul, exp on ACT,
    multiplicative causal mask on diagonal tiles, PV with ones-column rowsums,
    partial o_proj output [2048, 1024].
Host: sums the 4 per-head-group partials per batch (row-sharded o_proj
unshard) and stacks the 2 batches.
"""

import os
import sys

import numpy as np

sys.path.insert(0, "/opt/trn_rl_repo")

import concourse.bass as bass  # noqa: E402
import concourse.tile as tile  # noqa: E402
from concourse import bacc, mybir  # noqa: E402
from concourse import bass_utils  # noqa: E402

B, S, D, H, DK = 2, 2048, 1024, 16, 64
NCORES = 8
HPC = H // 4  # 4 heads per core
CW = HPC * DK  # 256 head-dim columns per core
VW = DK + 1  # 65: v width per head incl ones column
ND = D // 128  # 8 contraction chunks
NS = S // 128  # 16 s-tiles
NSC = S // 512  # 4 s-chunks
ROPE_THETA = 10000.0

F32 = mybir.dt.float32
F32R = mybir.dt.float32r
EXP = mybir.ActivationFunctionType.Exp


def _build_kernel(tc, nc, xt, wq, wk, wv, wo, cs, sn, mk, ones, out):
    from contextlib import ExitStack
    _stack = ExitStack()
    constp = _stack.enter_context(tc.tile_pool(name="const", bufs=1))
    pers = _stack.enter_context(tc.tile_pool(name="persist", bufs=1))

    wq_sb = constp.tile([128, ND * CW], F32R)
    wk_sb = constp.tile([128, ND * CW], F32R)
    wv_sb = constp.tile([128, ND * CW], F32R)
    wo_sb = constp.tile([128, 2 * D], F32R)
    cs_sb = constp.tile([128, S], F32)
    sn_sb = constp.tile([128, S], F32)
    mk_sb = constp.tile([128, 4 * 512], F32)
    for d in range(ND):
        nc.sync.dma_start(wq_sb[:, d * CW:(d + 1) * CW], wq[d * 128:(d + 1) * 128, :])
        nc.sync.dma_start(wk_sb[:, d * CW:(d + 1) * CW], wk[d * 128:(d + 1) * 128, :])
        nc.sync.dma_start(wv_sb[:, d * CW:(d + 1) * CW], wv[d * 128:(d + 1) * 128, :])
    nc.sync.dma_start(wo_sb[:, 0:D], wo[0:128, :])
    nc.sync.dma_start(wo_sb[:, D:2 * D], wo[128:256, :])
    nc.sync.dma_start(cs_sb[:], cs[:])
    nc.sync.dma_start(sn_sb[:], sn[:])
    nc.sync.dma_start(mk_sb[:], mk[:])

    qe_sb = pers.tile([128, S], F32R)
    qo_sb = pers.tile([128, S], F32R)
    ke_sb = pers.tile([128, S], F32R)
    ko_sb = pers.tile([128, S], F32R)
    v_sb = pers.tile([128, NS * HPC * VW], F32R)
    ctx_sb = pers.tile([128, 2 * S], F32R)

    # ---- Phase 1: projections ----
    with tc.tile_pool(name="xt", bufs=1) as xtp, \
         tc.tile_pool(name="pjps", bufs=5, space="PSUM") as pjps, \
         tc.tile_pool(name="vps", bufs=2, space="PSUM") as vps:
        xts = xtp.tile([128, ND * S], F32R)
        for d in range(ND):
            nc.sync.dma_start(xts[:, d * S:(d + 1) * S], xt[d * 128:(d + 1) * 128, :])
        blocks = [(qe_sb, wq_sb, 0), (qo_sb, wq_sb, 128),
                  (ke_sb, wk_sb, 0), (ko_sb, wk_sb, 128)]
        for dst, wsb, co in blocks:
            ps = [pjps.tile([128, 512], F32, name=f"pjt{_i}", tag="pjt") for _i in range(NSC)]
            for d in range(ND):
                lhsT = wsb[:, d * CW + co: d * CW + co + 128]
                for sc in range(NSC):
                    nc.tensor.matmul(
                        ps[sc][:],
                        lhsT,
                        xts[:, d * S + sc * 512: d * S + (sc + 1) * 512],
                        start=(d == 0), stop=(d == ND - 1))
            for sc in range(NSC):
                nc.vector.tensor_copy(dst[:, sc * 512:(sc + 1) * 512], ps[sc][:])
        for sm in range(NS):
            pv = vps.tile([128, CW], F32)
            for d in range(ND):
                nc.tensor.matmul(
                    pv[:],
                    xts[:, d * S + sm * 128: d * S + sm * 128 + 128],
                    wv_sb[:, d * CW:(d + 1) * CW],
                    start=(d == 0), stop=(d == ND - 1))
            base = sm * HPC * VW
            dst3 = v_sb[:, base:base + HPC * VW].rearrange("p (h c) -> p h c", c=VW)
            nc.vector.tensor_copy(dst3[:, :, 0:DK],
                                  pv[:].rearrange("p (h c) -> p h c", c=DK))
            nc.sync.dma_start(dst3[:, :, DK:DK + 1],
                              ones[:, sm * HPC:(sm + 1) * HPC].rearrange(
                                  "p (h c) -> p h c", c=1))

    # ---- Phase 2: RoPE (interleaved, on even/odd permuted layout) ----
    with tc.tile_pool(name="ropet", bufs=1) as rtp:
        for (e, o) in [(qe_sb, qo_sb), (ke_sb, ko_sb)]:
            t1 = rtp.tile([128, S], F32, tag="t1")
            t2 = rtp.tile([128, S], F32, tag="t2")
            t3 = rtp.tile([128, S], F32, tag="t3")
            t4 = rtp.tile([128, S], F32, tag="t4")
            nc.vector.tensor_mul(t1[:], e[:], cs_sb[:])
            nc.vector.tensor_mul(t2[:], o[:], sn_sb[:])
            nc.vector.tensor_mul(t3[:], o[:], cs_sb[:])
            nc.vector.tensor_mul(t4[:], e[:], sn_sb[:])
            nc.vector.tensor_sub(e[:], t1[:], t2[:])
            nc.vector.tensor_add(o[:], t3[:], t4[:])

    # ---- Phase 3: attention per head ----
    with tc.tile_pool(name="sps", bufs=4, space="PSUM") as sps, \
         tc.tile_pool(name="cps", bufs=2, space="PSUM") as cps, \
         tc.tile_pool(name="expool", bufs=6) as exp_pool, \
         tc.tile_pool(name="smp", bufs=4) as smp:
        for h in range(HPC):
            r0 = h * 32
            tp = (r0, 0)
            for c in range(NSC):
                nsk = 4 * (c + 1)
                pctx = cps.tile([VW, 512], F32)
                exps = []
                DEPTH = 3

                def pv_mm(t, nsk=nsk, pctx=pctx, exps=exps, h=h):
                    vbase = t * HPC * VW + h * VW
                    nc.tensor.matmul(
                        pctx[:],
                        v_sb[:, vbase:vbase + VW],
                        exps[t][:],
                        start=(t == 0), stop=(t == nsk - 1),
                        skip_group_check=True)

                for t in range(nsk):
                    pscore = sps.tile([128, 512], F32)
                    nc.tensor.matmul(
                        pscore[:],
                        ke_sb[r0:r0 + 32, t * 128:(t + 1) * 128],
                        qe_sb[r0:r0 + 32, c * 512:(c + 1) * 512],
                        start=True, stop=False, tile_position=tp,
                        skip_group_check=True)
                    nc.tensor.matmul(
                        pscore[:],
                        ko_sb[r0:r0 + 32, t * 128:(t + 1) * 128],
                        qo_sb[r0:r0 + 32, c * 512:(c + 1) * 512],
                        start=False, stop=True, tile_position=tp,
                        skip_group_check=True)
                    et = exp_pool.tile([128, 512], F32R)
                    nc.scalar.activation(et[:], pscore[:], EXP, scale=0.125)
                    j = t - 4 * c
                    if j >= 0:
                        nc.vector.tensor_mul(et[:], et[:],
                                             mk_sb[:, j * 512:(j + 1) * 512])
                    exps.append(et)
                    if t >= DEPTH:
                        pv_mm(t - DEPTH)
                for t in range(max(0, nsk - DEPTH), nsk):
                    pv_mm(t)
                rc = smp.tile([1, 512], F32, tag="rc")
                nc.vector.reciprocal(rc[:], pctx[DK:DK + 1, :])
                rb = smp.tile([64, 512], F32, tag="rb")
                nc.gpsimd.partition_broadcast(rb[:], rc[:])
                dst = ctx_sb[(h % 2) * 64:(h % 2) * 64 + 64,
                             (h // 2) * S + c * 512:(h // 2) * S + (c + 1) * 512]
                nc.vector.tensor_mul(dst, pctx[0:DK, :], rb[:])

    # ---- Phase 4: o_proj (partial over this core's 256 head dims) ----
    with tc.tile_pool(name="ops", bufs=4, space="PSUM") as opsp, \
         tc.tile_pool(name="obuf", bufs=4) as obp:
        for sm in range(NS):
            pos = [opsp.tile([128, 512], F32, name=f"opt{_i}", tag="opt") for _i in range(2)]
            for cb in range(2):
                lhsT = ctx_sb[:, cb * S + sm * 128: cb * S + sm * 128 + 128]
                for do_ in range(2):
                    nc.tensor.matmul(
                        pos[do_][:],
                        lhsT,
                        wo_sb[:, cb * D + do_ * 512: cb * D + (do_ + 1) * 512],
                        start=(cb == 0), stop=(cb == 1))
            for do_ in range(2):
                ot = obp.tile([128, 512], F32)
                nc.scalar.copy(ot[:], pos[do_][:])
                nc.sync.dma_start(out[sm * 128:(sm + 1) * 128,
                                      do_ * 512:(do_ + 1) * 512], ot[:])
    _stack.close()


def build_nc():
    nc = bacc.Bacc("TRN2", target_bir_lowering=False, debug=False,
                   enable_asserts=False, num_devices=NCORES)
    xt = nc.dram_tensor("xt", [D, S], F32R, kind="ExternalInput").ap()
    wq = nc.dram_tensor("wq", [D, CW], F32R, kind="ExternalInput").ap()
    wk = nc.dram_tensor("wk", [D, CW], F32R, kind="ExternalInput").ap()
    wv = nc.dram_tensor("wv", [D, CW], F32R, kind="ExternalInput").ap()
    wo = nc.dram_tensor("wo", [CW, D], F32R, kind="ExternalInput").ap()
    cs = nc.dram_tensor("cs", [128, S], F32, kind="ExternalInput").ap()
    sn = nc.dram_tensor("sn", [128, S], F32, kind="ExternalInput").ap()
    mk = nc.dram_tensor("mk", [128, 4 * 512], F32, kind="ExternalInput").ap()
    ones = nc.dram_tensor("ones", [128, NS * HPC], F32R,
                          kind="ExternalInput").ap()
    out = nc.dram_tensor("out_partial", [S, D], F32, kind="ExternalOutput").ap()
    with tile.TileContext(nc) as tc:
        _build_kernel(tc, nc, xt, wq, wk, wv, wo, cs, sn, mk, ones, out)
    nc.compile()
    return nc


def make_in_maps(in_features, q_proj_weight, k_proj_weight, v_proj_weight,
                 o_proj_weight, token_positions):
    x = np.asarray(in_features, dtype=np.float32)
    wq = np.asarray(q_proj_weight, dtype=np.float32)
    wk = np.asarray(k_proj_weight, dtype=np.float32)
    wv = np.asarray(v_proj_weight, dtype=np.float32)
    wo = np.asarray(o_proj_weight, dtype=np.float32)
    pos = np.asarray(token_positions).astype(np.float64)

    inv = ROPE_THETA ** (-2.0 * np.arange(DK // 2, dtype=np.float64) / DK)
    ang = inv[:, None] * pos[None, :]  # [32, S]
    cs_full = np.tile(np.cos(ang), (HPC, 1)).astype(np.float32)
    sn_full = np.tile(np.sin(ang), (HPC, 1)).astype(np.float32)

    p = np.arange(128)[:, None]
    f = np.arange(512)[None, :]
    mk = np.concatenate([(f >= j * 128 + p).astype(np.float32)
                         for j in range(4)], axis=1)

    in_maps = []
    for c in range(NCORES):
        b, g = c // 4, c % 4
        cols = np.arange(g * CW, (g + 1) * CW)
        hcols = cols.reshape(HPC, DK)
        qcols = np.concatenate([hcols[:, 0::2].reshape(-1),
                                hcols[:, 1::2].reshape(-1)])
        in_maps.append({
            "xt": np.ascontiguousarray(x[b].T),
            "wq": np.ascontiguousarray(wq[qcols, :].T),
            "wk": np.ascontiguousarray(wk[qcols, :].T),
            "wv": np.ascontiguousarray(wv[cols, :].T),
            "wo": np.ascontiguousarray(wo[:, cols].T),
            "cs": cs_full,
            "sn": sn_full,
            "mk": mk,
            "ones": np.ones((128, NS * HPC), np.float32),
        })
    return in_maps


_NC_CACHE = []
last_exec_ns = None
last_res = None


def kernel(in_features, q_proj_weight, k_proj_weight, v_proj_weight,
           o_proj_weight, token_positions, d_model=1024, num_heads=16,
           **_ignored):
    global last_exec_ns, last_res
    assert int(d_model) == D and int(num_heads) == H
    in_maps = make_in_maps(in_features, q_proj_weight, k_proj_weight,
                           v_proj_weight, o_proj_weight, token_positions)
    if not _NC_CACHE:
        _NC_CACHE.append(build_nc())
    nc = _NC_CACHE[0]
    trace = bool(int(os.environ.get("KERNEL_TRACE", "0")))
    res = bass_utils.run_bass_kernel_spmd(nc, in_maps,
                                          core_ids=list(range(NCORES)),
                                          trace=trace)
    last_exec_ns = res.exec_time_ns
    last_res = res
    parts = [r["out_partial"].astype(np.float32) for r in res.results]
    out = np.stack([parts[0] + parts[1] + parts[2] + parts[3],
                    parts[4] + parts[5] + parts[6] + parts[7]])
    return out

